# revision 16
# baseline (speedup 1.0000x reference)
"""Batched Viterbi decode (CRF) on 8 Trainium2 NeuronCores.

Device computes the bit-exact fp32 t1 value history; host backtracks in
numpy (the em-after-max reorder is exact: max_j fl(fl(t1+A)+em) ==
fl(max_j fl(t1+A) + em) by monotonicity of rounding).

Per step, per quad-group of 4 seqs: bias-adds (A_s + t1col) on 2xACT +
1 fused GPSIMD pair-add, per-seq PE transpose into a PSUM quad, one DVE
max-reduce [128,512], one DVE em-add [128,4] into t1hist.  The two
groups' (reduce -> em-add) pairs share a single-buffered tmp tile: the
WAR dependency stops the Tile scheduler from batching both reduces
back-to-back on DVE (which would put group1's reduce on group0's
critical path) and keeps the groups anti-phased.  Transposes are queued
on the PE in add-engine FIFO-rank order so a late add never blocks an
earlier-ready transpose.  t1 history streams to DRAM in chunks.

Env knobs (defaults are the tuned config):
  V5_RED   reduce group size (4)
  V5_ADD   8-char engine string per seq: a=ACT, v=DVE tensor_scalar,
           g=GPSIMD ("aaggaagg"; adjacent g's in a group are fused)
  V5_EM    engine for the em-adds: v=DVE (default) or g=GPSIMD
  V5_F32R  2 = float32r-typed transpose path (default 0; the BIR
           verifier requires the whole producer chain typed fp32r)
"""

import os
from contextlib import ExitStack

import numpy as np

S = 128
T = 2048
NS = 8
N_CORES = 8
B = NS * N_CORES

RED = int(os.environ.get("V5_RED", "4"))
ADD_ENG = os.environ.get("V5_ADD", "aaggaagg")
EM_ENG = os.environ.get("V5_EM", "v")
# 0 = fp32 transposes; 2 = fp32r-typed transpose path (ident/sc/psum
# tensors declared float32r, engine reads/writes via fp32 bitcast)
F32R_MODE = int(os.environ.get("V5_F32R", "0"))

_CACHE = {}


def _build_forward():
    import concourse.bacc as bacc
    import concourse.mybir as mybir
    import concourse.tile as tile

    F32 = mybir.dt.float32
    F32R = mybir.dt.float32r
    nc = bacc.Bacc("TRN2", num_devices=N_CORES)
    trans_in = nc.dram_tensor("transitions", [NS, S + 1, S], F32, kind="ExternalInput")
    em_in = nc.dram_tensor("emissions", [NS, T, S], F32, kind="ExternalInput")
    ident_in = nc.dram_tensor("identity", [S, S], F32, kind="ExternalInput")
    t1_out = nc.dram_tensor("t1hist", [S, T * NS], F32, kind="ExternalOutput")

    NGRP = NS // RED  # reduce groups per step
    TPDT = F32R if F32R_MODE == 2 else F32

    with ExitStack() as ctx:
        trans_sb = ctx.enter_context(nc.sbuf_tensor([S, NS * S], F32))
        em_cols = ctx.enter_context(nc.sbuf_tensor([S, T * NS], F32))
        t1hist = ctx.enter_context(nc.sbuf_tensor([S, T * NS], F32))
        ident = ctx.enter_context(nc.sbuf_tensor([S, S], F32))
        start_sb = ctx.enter_context(nc.sbuf_tensor([S, NS], F32))
        em0_sb = ctx.enter_context(nc.sbuf_tensor([S, NS], F32))
        if F32R_MODE == 2:
            ident_r = ctx.enter_context(nc.sbuf_tensor([S, S], F32R))
        # 2 quad-sized PSUM tensors x 3-deep rotation = 6 banks; prologue
        # stage psum uses the remaining 2.
        psum_pp = [
            [
                ctx.enter_context(nc.psum_tensor(f"pspp{k}g{g}", [S, 4 * S], TPDT))
                for g in range(2)
            ]
            for k in range(3)
        ]

        with tile.TileContext(nc) as tc, ExitStack() as pctx:
            sc_pool = pctx.enter_context(tc.tile_pool(name="scores", bufs=3))
            tmp_pool = pctx.enter_context(tc.tile_pool(name="tmp", bufs=1))
            stage_pool = pctx.enter_context(tc.tile_pool(name="stage", bufs=4))
            pst_pool = pctx.enter_context(tc.tile_pool(name="pst", bufs=2, space="PSUM"))

            # ---- prologue: transitions, identity, t1_0 ----
            for s in range(NS):
                nc.sync.dma_start(trans_sb[:, s * S:(s + 1) * S], trans_in[s, 0:S, :])
            nc.sync.dma_start(ident[:], ident_in[:])
            if F32R_MODE == 2:
                nc.sync.dma_start(ident_r[:], ident_in[:].bitcast(F32R))
            for s in range(NS):
                nc.sync.dma_start(
                    start_sb[:, s:s + 1], trans_in[s, S:S + 1, :].rearrange("o p -> p o")
                )
                nc.sync.dma_start(
                    em0_sb[:, s:s + 1], em_in[s, 0:1, :].rearrange("o p -> p o")
                )
            nc.vector.tensor_add(t1hist[:, 0:NS], start_sb[:], em0_sb[:])

            # ---- prologue: transpose emissions into em_cols[i, t*NS+s] ----
            for s in range(NS):
                for q in range(T // S // 4):
                    stage = stage_pool.tile([S, 4 * S], F32, tag="emstage")
                    pst = pst_pool.tile([S, 4 * S], F32, tag="empsum")
                    for k in range(4):
                        c = 4 * q + k
                        nc.sync.dma_start(
                            stage[:, k * S:(k + 1) * S], em_in[s, c * S:(c + 1) * S, :])
                        nc.tensor.transpose(
                            pst[:, k * S:(k + 1) * S], stage[:, k * S:(k + 1) * S],
                            ident[:])
                    dst = em_cols[:, 4 * q * S * NS + s: 4 * (q + 1) * S * NS: NS]
                    nc.scalar.copy(dst, pst[:])

            # ---- main DP loop ----
            def emit_add(s, t1col, dst):
                src = trans_sb[:, s * S:(s + 1) * S]
                e = ADD_ENG[s]
                if e == "a":
                    nc.scalar.activation(
                        dst, src, mybir.ActivationFunctionType.Identity,
                        bias=t1col, scale=1.0,
                    )
                elif e == "v":
                    nc.vector.tensor_scalar_add(dst, src, t1col)
                else:  # g
                    nc.gpsimd.tensor_add(
                        dst.rearrange("p (o i) -> p o i", i=S),
                        src.rearrange("p (o i) -> p o i", i=S),
                        t1col[:, :, None].to_broadcast([S, 1, S]))

            def emit_tiny(t, s0, tmp):
                # t1hist[:, t*NS+s0 : +RED] = u + em  (group granularity)
                lo = t * NS + s0
                args = (t1hist[:, lo:lo + RED], tmp[:],
                        em_cols[:, lo:lo + RED])
                if EM_ENG == "g":
                    nc.gpsimd.tensor_add(*args)
                else:
                    nc.vector.tensor_add(*args)

            def step(t):
                base = (t - 1) * NS
                for grp in range(NGRP):
                    s0 = grp * RED
                    q = (s0 // 4)
                    pst = psum_pp[t % 3][q]
                    half = (s0 % 4)
                    # emit ACT adds first: they serialize on the Scalar
                    # engine, so release them as early as possible
                    order = sorted(range(RED),
                                   key=lambda sl: ADD_ENG[s0 + sl] != "a")
                    # fuse two consecutive GPSIMD slots into one wide pair
                    # add: GPSIMD op cost is fixed-overhead dominated, so one
                    # [128,2,128] op beats two serialized [128,128] ops
                    gs = [sl for sl in order if ADD_ENG[s0 + sl] == "g"]
                    fused = ()
                    if len(gs) == 2 and abs(gs[0] - gs[1]) == 1:
                        fused = (min(gs), max(gs))
                    sc_aps = {}
                    ranks = {}
                    nseen = {}
                    for sl in order:
                        s = s0 + sl
                        if fused and sl == fused[1]:
                            continue
                        if fused and sl == fused[0]:
                            scp = sc_pool.tile([S, 2 * S], TPDT, tag=f"scp{s}")
                            dstp = (scp[:].bitcast(F32)
                                    if F32R_MODE == 2 else scp[:])
                            nc.gpsimd.tensor_add(
                                dstp.rearrange("p (o i) -> p o i", i=S),
                                trans_sb[:, s * S:(s + 2) * S].rearrange(
                                    "p (o i) -> p o i", i=S),
                                t1hist[:, base + s:base + s + 2, None]
                                .to_broadcast([S, 2, S]))
                            sc_aps[fused[0]] = scp[:, 0:S]
                            sc_aps[fused[1]] = scp[:, S:2 * S]
                            ranks[fused[0]] = 0
                            ranks[fused[1]] = 0.5
                            continue
                        sc = sc_pool.tile([S, S], TPDT, tag=f"sc{s}")
                        dst = sc[:].bitcast(F32) if F32R_MODE == 2 else sc[:]
                        emit_add(s, t1hist[:, base + s:base + s + 1], dst)
                        sc_aps[sl] = sc[:]
                        e = ADD_ENG[s]
                        ranks[sl] = nseen.get(e, 0)
                        nseen[e] = ranks[sl] + 1
                    # transposes in expected ready-time order: a slot's add
                    # finishes after the other adds queued before it on the
                    # same engine, so interleave engines by FIFO rank to
                    # avoid head-of-line blocking in the PE queue
                    tp_order = sorted(order, key=lambda sl: ranks[sl])
                    for sl in tp_order:
                        nc.tensor.transpose(
                            pst[:, (half + sl) * S:(half + sl + 1) * S],
                            sc_aps[sl],
                            ident_r[:] if F32R_MODE == 2 else ident[:])
                    pgsrc = pst[:, half * S:(half + RED) * S]
                    if F32R_MODE == 2:
                        pgsrc = pgsrc.bitcast(F32)
                    pg = pgsrc.rearrange("p (s i) -> p s i", i=S)
                    # single-buffered tmp shared by both groups: the WAR
                    # dependency forces the scheduler to keep each group's
                    # reduce->tiny adjacent on DVE instead of batching both
                    # reduces first (which puts group1's reduce on group0's
                    # critical path)
                    tmp = tmp_pool.tile([S, RED], F32, tag="u")
                    nc.vector.tensor_reduce(
                        tmp[:], pg,
                        axis=mybir.AxisListType.X, op=mybir.AluOpType.max)
                    emit_tiny(t, s0, tmp)

            CHUNK = 512
            for t in range(1, T):
                step(t)
                if t % CHUNK == 0:
                    lo = (t - CHUNK) * NS
                    nc.sync.dma_start(
                        t1_out[:, lo:t * NS], t1hist[:, lo:t * NS])

            lo = (T // CHUNK * CHUNK - CHUNK) * NS
            nc.sync.dma_start(t1_out[:, lo:], t1hist[:, lo:])

    nc.finalize()
    return nc


def _get_nc():
    if "nc" not in _CACHE:
        _CACHE["nc"] = _build_forward()
    return _CACHE["nc"]


def kernel(transitions, emissions, lengths):
    from concourse.bass_utils import run_bass_kernel_spmd

    transitions = np.ascontiguousarray(transitions, dtype=np.float32)
    emissions = np.ascontiguousarray(emissions, dtype=np.float32)
    lengths = np.asarray(lengths, dtype=np.int32)
    assert transitions.shape == (B, S + 1, S)
    assert emissions.shape == (B, T, S)

    nc = _get_nc()
    eye = np.eye(S, dtype=np.float32)
    in_maps = [
        {
            "transitions": transitions[c * NS:(c + 1) * NS],
            "emissions": emissions[c * NS:(c + 1) * NS],
            "identity": eye,
        }
        for c in range(N_CORES)
    ]
    res = run_bass_kernel_spmd(
        nc, in_maps, core_ids=list(range(N_CORES)),
        trace=bool(os.environ.get("VIT_TRACE")),
    )
    if os.environ.get("VIT_TRACE"):
        _CACHE["last_exec_time_ns"] = res.exec_time_ns
        _CACHE["last_res"] = res

    t1 = np.empty((B, T, S), dtype=np.float32)
    for c in range(N_CORES):
        t1[c * NS:(c + 1) * NS] = (
            res.results[c]["t1hist"].reshape(S, T, NS).transpose(2, 1, 0)
        )

    return _backtrack(transitions, emissions, lengths, t1)


def _backtrack(transitions, emissions, lengths, t1):
    """Reference-exact backtrack from the t1 value history."""
    trans = transitions[:, :S, :]
    nb = np.arange(B)
    z = np.zeros((B, T), dtype=np.int32)
    last = lengths - 1
    z_last = np.argmax(t1[nb, last, :], axis=1).astype(np.int32)
    ptr = z_last.copy()
    for t in range(int(last.max()), 0, -1):
        at_last = (t == last)
        if at_last.any():
            ptr = np.where(at_last, z_last, ptr)
        z[:, t] = np.where(t <= last, ptr, 0)
        col = (t1[:, t - 1, :] + trans[nb, :, ptr]) + emissions[nb, t, ptr][:, None]
        ptr_new = np.argmax(col, axis=1).astype(np.int32)
        ptr = np.where(t <= last, ptr_new, ptr)
    z[:, 0] = ptr
    return z


# revision 18
# speedup vs baseline: 1.0061x; 1.0061x over previous
"""Batched Viterbi decode (CRF) on 8 Trainium2 NeuronCores.

Device computes the bit-exact fp32 t1 value history; host backtracks in
numpy (the em-after-max reorder is exact: max_j fl(fl(t1+A)+em) ==
fl(max_j fl(t1+A) + em) by monotonicity of rounding).

Per step, per quad-group of 4 seqs: bias-adds (A_s + t1col) on 2xACT +
1 fused GPSIMD pair-add, per-seq PE transpose into a PSUM quad, one DVE
max-reduce [128,512], one DVE em-add [128,4] into t1hist.  The two
groups' (reduce -> em-add) pairs share a single-buffered tmp tile: the
WAR dependency stops the Tile scheduler from batching both reduces
back-to-back on DVE (which would put group1's reduce on group0's
critical path) and keeps the groups anti-phased.  Transposes are queued
on the PE in add-engine FIFO-rank order so a late add never blocks an
earlier-ready transpose.  t1 history streams to DRAM in chunks.

Env knobs (defaults are the tuned config):
  V5_RED   reduce group size (4)
  V5_ADD   8-char engine string per seq: a=ACT, v=DVE tensor_scalar,
           g=GPSIMD ("aaggaagg"; adjacent g's in a group are fused)
  V5_EM    engine for the em-adds: v=DVE (default) or g=GPSIMD
  V5_F32R  2 = float32r-typed transpose path (default 0; the BIR
           verifier requires the whole producer chain typed fp32r)
"""

import os
from contextlib import ExitStack

import numpy as np

S = 128
T = 2048
NS = 8
N_CORES = 8
B = NS * N_CORES

RED = int(os.environ.get("V5_RED", "4"))
ADD_ENG = os.environ.get("V5_ADD", "aaggaagg")
EM_ENG = os.environ.get("V5_EM", "v")
# 0 = fp32 transposes; 2 = fp32r-typed transpose path (ident/sc/psum
# tensors declared float32r, engine reads/writes via fp32 bitcast)
F32R_MODE = int(os.environ.get("V5_F32R", "0"))
# fuse adjacent GPSIMD adds into one wide op (measured slower: the
# single op gates both transposes on its full completion)
GFUSE = os.environ.get("V5_GFUSE", "0") == "1"

_CACHE = {}


def _build_forward():
    import concourse.bacc as bacc
    import concourse.mybir as mybir
    import concourse.tile as tile

    F32 = mybir.dt.float32
    F32R = mybir.dt.float32r
    nc = bacc.Bacc("TRN2", num_devices=N_CORES)
    trans_in = nc.dram_tensor("transitions", [NS, S + 1, S], F32, kind="ExternalInput")
    em_in = nc.dram_tensor("emissions", [NS, T, S], F32, kind="ExternalInput")
    ident_in = nc.dram_tensor("identity", [S, S], F32, kind="ExternalInput")
    t1_out = nc.dram_tensor("t1hist", [S, T * NS], F32, kind="ExternalOutput")

    NGRP = NS // RED  # reduce groups per step
    TPDT = F32R if F32R_MODE == 2 else F32

    with ExitStack() as ctx:
        trans_sb = ctx.enter_context(nc.sbuf_tensor([S, NS * S], F32))
        em_cols = ctx.enter_context(nc.sbuf_tensor([S, T * NS], F32))
        t1hist = ctx.enter_context(nc.sbuf_tensor([S, T * NS], F32))
        ident = ctx.enter_context(nc.sbuf_tensor([S, S], F32))
        start_sb = ctx.enter_context(nc.sbuf_tensor([S, NS], F32))
        em0_sb = ctx.enter_context(nc.sbuf_tensor([S, NS], F32))
        if F32R_MODE == 2:
            ident_r = ctx.enter_context(nc.sbuf_tensor([S, S], F32R))
        # 2 quad-sized PSUM tensors x 3-deep rotation = 6 banks; prologue
        # stage psum uses the remaining 2.
        psum_pp = [
            [
                ctx.enter_context(nc.psum_tensor(f"pspp{k}g{g}", [S, 4 * S], TPDT))
                for g in range(2)
            ]
            for k in range(3)
        ]

        with tile.TileContext(nc) as tc, ExitStack() as pctx:
            sc_pool = pctx.enter_context(tc.tile_pool(name="scores", bufs=3))
            tmp_pool = pctx.enter_context(tc.tile_pool(name="tmp", bufs=1))
            stage_pool = pctx.enter_context(tc.tile_pool(name="stage", bufs=4))
            pst_pool = pctx.enter_context(tc.tile_pool(name="pst", bufs=2, space="PSUM"))

            # ---- prologue: transitions, identity, t1_0 ----
            for s in range(NS):
                nc.sync.dma_start(trans_sb[:, s * S:(s + 1) * S], trans_in[s, 0:S, :])
            nc.sync.dma_start(ident[:], ident_in[:])
            if F32R_MODE == 2:
                nc.sync.dma_start(ident_r[:], ident_in[:].bitcast(F32R))
            for s in range(NS):
                nc.sync.dma_start(
                    start_sb[:, s:s + 1], trans_in[s, S:S + 1, :].rearrange("o p -> p o")
                )
                nc.sync.dma_start(
                    em0_sb[:, s:s + 1], em_in[s, 0:1, :].rearrange("o p -> p o")
                )
            nc.vector.tensor_add(t1hist[:, 0:NS], start_sb[:], em0_sb[:])

            # ---- prologue: transpose emissions into em_cols[i, t*NS+s] ----
            for s in range(NS):
                for q in range(T // S // 4):
                    stage = stage_pool.tile([S, 4 * S], F32, tag="emstage")
                    pst = pst_pool.tile([S, 4 * S], F32, tag="empsum")
                    for k in range(4):
                        c = 4 * q + k
                        nc.sync.dma_start(
                            stage[:, k * S:(k + 1) * S], em_in[s, c * S:(c + 1) * S, :])
                        nc.tensor.transpose(
                            pst[:, k * S:(k + 1) * S], stage[:, k * S:(k + 1) * S],
                            ident[:])
                    dst = em_cols[:, 4 * q * S * NS + s: 4 * (q + 1) * S * NS: NS]
                    nc.scalar.copy(dst, pst[:])

            # ---- main DP loop ----
            def emit_add(s, t1col, dst):
                src = trans_sb[:, s * S:(s + 1) * S]
                e = ADD_ENG[s]
                if e == "a":
                    nc.scalar.activation(
                        dst, src, mybir.ActivationFunctionType.Identity,
                        bias=t1col, scale=1.0,
                    )
                elif e == "v":
                    nc.vector.tensor_scalar_add(dst, src, t1col)
                else:  # g
                    nc.gpsimd.tensor_add(
                        dst.rearrange("p (o i) -> p o i", i=S),
                        src.rearrange("p (o i) -> p o i", i=S),
                        t1col[:, :, None].to_broadcast([S, 1, S]))

            def emit_tiny(t, s0, tmp):
                # t1hist[:, t*NS+s0 : +RED] = u + em  (group granularity)
                lo = t * NS + s0
                args = (t1hist[:, lo:lo + RED], tmp[:],
                        em_cols[:, lo:lo + RED])
                if EM_ENG == "g":
                    nc.gpsimd.tensor_add(*args)
                else:
                    nc.vector.tensor_add(*args)

            def step(t):
                base = (t - 1) * NS
                for grp in range(NGRP):
                    s0 = grp * RED
                    q = (s0 // 4)
                    pst = psum_pp[t % 3][q]
                    half = (s0 % 4)
                    # emit ACT adds first: they serialize on the Scalar
                    # engine, so release them as early as possible
                    order = sorted(range(RED),
                                   key=lambda sl: ADD_ENG[s0 + sl] != "a")
                    # fuse two consecutive GPSIMD slots into one wide pair
                    # add: GPSIMD op cost is fixed-overhead dominated, so one
                    # [128,2,128] op beats two serialized [128,128] ops
                    gs = [sl for sl in order if ADD_ENG[s0 + sl] == "g"]
                    fused = ()
                    if (GFUSE and len(gs) == 2 and abs(gs[0] - gs[1]) == 1):
                        fused = (min(gs), max(gs))
                    sc_aps = {}
                    ranks = {}
                    nseen = {}
                    for sl in order:
                        s = s0 + sl
                        if fused and sl == fused[1]:
                            continue
                        if fused and sl == fused[0]:
                            scp = sc_pool.tile([S, 2 * S], TPDT, tag=f"scp{s}")
                            dstp = (scp[:].bitcast(F32)
                                    if F32R_MODE == 2 else scp[:])
                            nc.gpsimd.tensor_add(
                                dstp.rearrange("p (o i) -> p o i", i=S),
                                trans_sb[:, s * S:(s + 2) * S].rearrange(
                                    "p (o i) -> p o i", i=S),
                                t1hist[:, base + s:base + s + 2, None]
                                .to_broadcast([S, 2, S]))
                            sc_aps[fused[0]] = scp[:, 0:S]
                            sc_aps[fused[1]] = scp[:, S:2 * S]
                            ranks[fused[0]] = 0
                            ranks[fused[1]] = 0.5
                            continue
                        sc = sc_pool.tile([S, S], TPDT, tag=f"sc{s}")
                        dst = sc[:].bitcast(F32) if F32R_MODE == 2 else sc[:]
                        emit_add(s, t1hist[:, base + s:base + s + 1], dst)
                        sc_aps[sl] = sc[:]
                        e = ADD_ENG[s]
                        ranks[sl] = nseen.get(e, 0)
                        nseen[e] = ranks[sl] + 1
                    # transposes in expected ready-time order: a slot's add
                    # finishes after the other adds queued before it on the
                    # same engine, so interleave engines by FIFO rank to
                    # avoid head-of-line blocking in the PE queue
                    tp_order = sorted(order, key=lambda sl: ranks[sl])
                    for sl in tp_order:
                        nc.tensor.transpose(
                            pst[:, (half + sl) * S:(half + sl + 1) * S],
                            sc_aps[sl],
                            ident_r[:] if F32R_MODE == 2 else ident[:])
                    pgsrc = pst[:, half * S:(half + RED) * S]
                    if F32R_MODE == 2:
                        pgsrc = pgsrc.bitcast(F32)
                    pg = pgsrc.rearrange("p (s i) -> p s i", i=S)
                    # single-buffered tmp shared by both groups: the WAR
                    # dependency forces the scheduler to keep each group's
                    # reduce->tiny adjacent on DVE instead of batching both
                    # reduces first (which puts group1's reduce on group0's
                    # critical path)
                    tmp = tmp_pool.tile([S, RED], F32, tag="u")
                    nc.vector.tensor_reduce(
                        tmp[:], pg,
                        axis=mybir.AxisListType.X, op=mybir.AluOpType.max)
                    emit_tiny(t, s0, tmp)

            CHUNK = 512
            for t in range(1, T):
                step(t)
                if t % CHUNK == 0:
                    lo = (t - CHUNK) * NS
                    nc.sync.dma_start(
                        t1_out[:, lo:t * NS], t1hist[:, lo:t * NS])

            lo = (T // CHUNK * CHUNK - CHUNK) * NS
            nc.sync.dma_start(t1_out[:, lo:], t1hist[:, lo:])

    nc.finalize()
    return nc


def _get_nc():
    if "nc" not in _CACHE:
        _CACHE["nc"] = _build_forward()
    return _CACHE["nc"]


def kernel(transitions, emissions, lengths):
    from concourse.bass_utils import run_bass_kernel_spmd

    transitions = np.ascontiguousarray(transitions, dtype=np.float32)
    emissions = np.ascontiguousarray(emissions, dtype=np.float32)
    lengths = np.asarray(lengths, dtype=np.int32)
    assert transitions.shape == (B, S + 1, S)
    assert emissions.shape == (B, T, S)

    nc = _get_nc()
    eye = np.eye(S, dtype=np.float32)
    in_maps = [
        {
            "transitions": transitions[c * NS:(c + 1) * NS],
            "emissions": emissions[c * NS:(c + 1) * NS],
            "identity": eye,
        }
        for c in range(N_CORES)
    ]
    res = run_bass_kernel_spmd(
        nc, in_maps, core_ids=list(range(N_CORES)),
        trace=bool(os.environ.get("VIT_TRACE")),
    )
    if os.environ.get("VIT_TRACE"):
        _CACHE["last_exec_time_ns"] = res.exec_time_ns
        _CACHE["last_res"] = res

    t1 = np.empty((B, T, S), dtype=np.float32)
    for c in range(N_CORES):
        t1[c * NS:(c + 1) * NS] = (
            res.results[c]["t1hist"].reshape(S, T, NS).transpose(2, 1, 0)
        )

    return _backtrack(transitions, emissions, lengths, t1)


def _backtrack(transitions, emissions, lengths, t1):
    """Reference-exact backtrack from the t1 value history."""
    trans = transitions[:, :S, :]
    nb = np.arange(B)
    z = np.zeros((B, T), dtype=np.int32)
    last = lengths - 1
    z_last = np.argmax(t1[nb, last, :], axis=1).astype(np.int32)
    ptr = z_last.copy()
    for t in range(int(last.max()), 0, -1):
        at_last = (t == last)
        if at_last.any():
            ptr = np.where(at_last, z_last, ptr)
        z[:, t] = np.where(t <= last, ptr, 0)
        col = (t1[:, t - 1, :] + trans[nb, :, ptr]) + emissions[nb, t, ptr][:, None]
        ptr_new = np.argmax(col, axis=1).astype(np.int32)
        ptr = np.where(t <= last, ptr_new, ptr)
    z[:, 0] = ptr
    return z


# revision 21
# speedup vs baseline: 1.0473x; 1.0410x over previous
"""Batched Viterbi decode (CRF) on 8 Trainium2 NeuronCores.

Device computes the bit-exact fp32 t1 value history; host backtracks in
numpy (the em-after-max reorder is exact: max_j fl(fl(t1+A)+em) ==
fl(max_j fl(t1+A) + em) by monotonicity of rounding).

Per step, per quad-group of 4 seqs: bias-adds (A_s + t1col) on 2xACT +
1 fused GPSIMD pair-add, per-seq PE transpose into a PSUM quad, one DVE
max-reduce [128,512], one DVE em-add [128,4] into t1hist.  The two
groups' (reduce -> em-add) pairs share a single-buffered tmp tile: the
WAR dependency stops the Tile scheduler from batching both reduces
back-to-back on DVE (which would put group1's reduce on group0's
critical path) and keeps the groups anti-phased.  Transposes are queued
on the PE in add-engine FIFO-rank order so a late add never blocks an
earlier-ready transpose.  t1 history streams to DRAM in chunks.

Env knobs (defaults are the tuned config):
  V5_RED   reduce group size (4)
  V5_ADD   8-char engine string per seq: a=ACT, v=DVE tensor_scalar,
           g=GPSIMD ("aaggaagg"; adjacent g's in a group are fused)
  V5_EM    engine for the em-adds: v=DVE (default) or g=GPSIMD
  V5_F32R  2 = float32r-typed transpose path (default 0; the BIR
           verifier requires the whole producer chain typed fp32r)
"""

import os
from contextlib import ExitStack

import numpy as np

S = 128
T = 2048
NS = 8
N_CORES = 8
B = NS * N_CORES

RED = int(os.environ.get("V5_RED", "4"))
ADD_ENG = os.environ.get("V5_ADD", "aaggaagg")
EM_ENG = os.environ.get("V5_EM", "v")
# 0 = fp32 transposes; 2 = fp32r-typed transpose path (ident/sc/psum
# tensors declared float32r, engine reads/writes via fp32 bitcast)
F32R_MODE = int(os.environ.get("V5_F32R", "0"))
# fuse adjacent GPSIMD adds into one wide op (measured slower: the
# single op gates both transposes on its full completion)
GFUSE = os.environ.get("V5_GFUSE", "0") == "1"
# split each group's reduce: early 3-slice tensor_reduce + a fused
# tensor_tensor_reduce on the last-ready slice that adds em and writes
# the t1 column directly, unblocking the next step's adds earlier
TTR = os.environ.get("V5_TTR", "0") == "1"  # crashes NRT at runtime on this hw

_CACHE = {}


def _build_forward():
    import concourse.bacc as bacc
    import concourse.mybir as mybir
    import concourse.tile as tile

    F32 = mybir.dt.float32
    F32R = mybir.dt.float32r
    nc = bacc.Bacc("TRN2", num_devices=N_CORES)
    trans_in = nc.dram_tensor("transitions", [NS, S + 1, S], F32, kind="ExternalInput")
    em_in = nc.dram_tensor("emissions", [NS, T, S], F32, kind="ExternalInput")
    ident_in = nc.dram_tensor("identity", [S, S], F32, kind="ExternalInput")
    t1_out = nc.dram_tensor("t1hist", [S, T * NS], F32, kind="ExternalOutput")

    NGRP = NS // RED  # reduce groups per step
    TPDT = F32R if F32R_MODE == 2 else F32

    with ExitStack() as ctx:
        trans_sb = ctx.enter_context(nc.sbuf_tensor([S, NS * S], F32))
        em_cols = ctx.enter_context(nc.sbuf_tensor([S, T * NS], F32))
        t1hist = ctx.enter_context(nc.sbuf_tensor([S, T * NS], F32))
        ident = ctx.enter_context(nc.sbuf_tensor([S, S], F32))
        start_sb = ctx.enter_context(nc.sbuf_tensor([S, NS], F32))
        em0_sb = ctx.enter_context(nc.sbuf_tensor([S, NS], F32))
        if F32R_MODE == 2:
            ident_r = ctx.enter_context(nc.sbuf_tensor([S, S], F32R))
        # 2 quad-sized PSUM tensors x 3-deep rotation = 6 banks; prologue
        # stage psum uses the remaining 2.
        psum_pp = [
            [
                ctx.enter_context(nc.psum_tensor(f"pspp{k}g{g}", [S, 4 * S], TPDT))
                for g in range(2)
            ]
            for k in range(3)
        ]

        with tile.TileContext(nc) as tc, ExitStack() as pctx:
            sc_pool = pctx.enter_context(tc.tile_pool(name="scores", bufs=3))
            tmp_pool = pctx.enter_context(tc.tile_pool(name="tmp", bufs=1))
            stage_pool = pctx.enter_context(tc.tile_pool(name="stage", bufs=4))
            pst_pool = pctx.enter_context(tc.tile_pool(name="pst", bufs=2, space="PSUM"))

            # ---- prologue: transitions, identity, t1_0 ----
            for s in range(NS):
                nc.sync.dma_start(trans_sb[:, s * S:(s + 1) * S], trans_in[s, 0:S, :])
            nc.sync.dma_start(ident[:], ident_in[:])
            if F32R_MODE == 2:
                nc.sync.dma_start(ident_r[:], ident_in[:].bitcast(F32R))
            for s in range(NS):
                nc.sync.dma_start(
                    start_sb[:, s:s + 1], trans_in[s, S:S + 1, :].rearrange("o p -> p o")
                )
                nc.sync.dma_start(
                    em0_sb[:, s:s + 1], em_in[s, 0:1, :].rearrange("o p -> p o")
                )
            nc.vector.tensor_add(t1hist[:, 0:NS], start_sb[:], em0_sb[:])

            # ---- prologue: transpose emissions into em_cols[i, t*NS+s] ----
            for s in range(NS):
                for q in range(T // S // 4):
                    stage = stage_pool.tile([S, 4 * S], F32, tag="emstage")
                    pst = pst_pool.tile([S, 4 * S], F32, tag="empsum")
                    for k in range(4):
                        c = 4 * q + k
                        nc.sync.dma_start(
                            stage[:, k * S:(k + 1) * S], em_in[s, c * S:(c + 1) * S, :])
                        nc.tensor.transpose(
                            pst[:, k * S:(k + 1) * S], stage[:, k * S:(k + 1) * S],
                            ident[:])
                    dst = em_cols[:, 4 * q * S * NS + s: 4 * (q + 1) * S * NS: NS]
                    nc.scalar.copy(dst, pst[:])

            # ---- main DP loop ----
            def emit_add(s, t1col, dst):
                src = trans_sb[:, s * S:(s + 1) * S]
                e = ADD_ENG[s]
                if e == "a":
                    nc.scalar.activation(
                        dst, src, mybir.ActivationFunctionType.Identity,
                        bias=t1col, scale=1.0,
                    )
                elif e == "v":
                    nc.vector.tensor_scalar_add(dst, src, t1col)
                else:  # g
                    nc.gpsimd.tensor_add(
                        dst.rearrange("p (o i) -> p o i", i=S),
                        src.rearrange("p (o i) -> p o i", i=S),
                        t1col[:, :, None].to_broadcast([S, 1, S]))

            def emit_tiny(t, s0, tmp):
                # t1hist[:, t*NS+s0 : +RED] = u + em  (group granularity)
                lo = t * NS + s0
                args = (t1hist[:, lo:lo + RED], tmp[:],
                        em_cols[:, lo:lo + RED])
                if EM_ENG == "g":
                    nc.gpsimd.tensor_add(*args)
                else:
                    nc.vector.tensor_add(*args)

            def step(t):
                base = (t - 1) * NS
                for grp in range(NGRP):
                    s0 = grp * RED
                    q = (s0 // 4)
                    pst = psum_pp[t % 3][q]
                    half = (s0 % 4)
                    # emit ACT adds first: they serialize on the Scalar
                    # engine, so release them as early as possible
                    order = sorted(range(RED),
                                   key=lambda sl: ADD_ENG[s0 + sl] != "a")
                    # fuse two consecutive GPSIMD slots into one wide pair
                    # add: GPSIMD op cost is fixed-overhead dominated, so one
                    # [128,2,128] op beats two serialized [128,128] ops
                    gs = [sl for sl in order if ADD_ENG[s0 + sl] == "g"]
                    fused = ()
                    if (GFUSE and len(gs) == 2 and abs(gs[0] - gs[1]) == 1):
                        fused = (min(gs), max(gs))
                    sc_aps = {}
                    ranks = {}
                    nseen = {}
                    for sl in order:
                        s = s0 + sl
                        if fused and sl == fused[1]:
                            continue
                        if fused and sl == fused[0]:
                            scp = sc_pool.tile([S, 2 * S], TPDT, tag=f"scp{s}")
                            dstp = (scp[:].bitcast(F32)
                                    if F32R_MODE == 2 else scp[:])
                            nc.gpsimd.tensor_add(
                                dstp.rearrange("p (o i) -> p o i", i=S),
                                trans_sb[:, s * S:(s + 2) * S].rearrange(
                                    "p (o i) -> p o i", i=S),
                                t1hist[:, base + s:base + s + 2, None]
                                .to_broadcast([S, 2, S]))
                            sc_aps[fused[0]] = scp[:, 0:S]
                            sc_aps[fused[1]] = scp[:, S:2 * S]
                            ranks[fused[0]] = 0
                            ranks[fused[1]] = 0.5
                            continue
                        sc = sc_pool.tile([S, S], TPDT, tag=f"sc{s}")
                        dst = sc[:].bitcast(F32) if F32R_MODE == 2 else sc[:]
                        emit_add(s, t1hist[:, base + s:base + s + 1], dst)
                        sc_aps[sl] = sc[:]
                        e = ADD_ENG[s]
                        ranks[sl] = nseen.get(e, 0)
                        nseen[e] = ranks[sl] + 1
                    # transposes in expected ready-time order: a slot's add
                    # finishes after the other adds queued before it on the
                    # same engine, so interleave engines by FIFO rank to
                    # avoid head-of-line blocking in the PE queue
                    tp_order = sorted(order, key=lambda sl: ranks[sl])
                    for sl in tp_order:
                        nc.tensor.transpose(
                            pst[:, (half + sl) * S:(half + sl + 1) * S],
                            sc_aps[sl],
                            ident_r[:] if F32R_MODE == 2 else ident[:])
                    lo = t * NS + s0
                    if TTR and RED == 4 and tp_order[-1] == 3:
                        # early 3 slices: reduce + em-add as soon as the
                        # first three transposes land; the single-buffered
                        # tmp (WAR with the other group) keeps reduce->add
                        # adjacent on DVE and the groups anti-phased
                        pg3 = pst[:, half * S:(half + 3) * S]
                        if F32R_MODE == 2:
                            pg3 = pg3.bitcast(F32)
                        pg3 = pg3.rearrange("p (s i) -> p s i", i=S)
                        tmp = tmp_pool.tile([S, 3], F32, tag="u")
                        nc.vector.tensor_reduce(
                            tmp[:], pg3,
                            axis=mybir.AxisListType.X, op=mybir.AluOpType.max)
                        nc.vector.tensor_add(
                            t1hist[:, lo:lo + 3], tmp[:], em_cols[:, lo:lo + 3])
                        # last-ready slice: fused (psum + em) -> max writes
                        # the t1 column directly (reference rounding order)
                        p1 = pst[:, (half + 3) * S:(half + 4) * S]
                        if F32R_MODE == 2:
                            p1 = p1.bitcast(F32)
                        scr = sc_pool.tile([S, S], F32, tag=f"ttr{grp}")
                        nc.vector.tensor_tensor_reduce(
                            out=scr[:].rearrange("p (o i) -> p o i", i=S),
                            in0=p1.rearrange("p (o i) -> p o i", i=S),
                            in1=em_cols[:, lo + 3:lo + 4, None]
                            .to_broadcast([S, 1, S]),
                            scale=1.0, scalar=float(-3.4e38),
                            op0=mybir.AluOpType.add, op1=mybir.AluOpType.max,
                            accum_out=t1hist[:, lo + 3:lo + 4])
                    else:
                        pgsrc = pst[:, half * S:(half + RED) * S]
                        if F32R_MODE == 2:
                            pgsrc = pgsrc.bitcast(F32)
                        pg = pgsrc.rearrange("p (s i) -> p s i", i=S)
                        # single-buffered tmp shared by both groups: the WAR
                        # dependency forces the scheduler to keep each group's
                        # reduce->tiny adjacent on DVE instead of batching both
                        # reduces first (which puts group1's reduce on group0's
                        # critical path)
                        tmp = tmp_pool.tile([S, RED], F32, tag="u")
                        nc.vector.tensor_reduce(
                            tmp[:], pg,
                            axis=mybir.AxisListType.X, op=mybir.AluOpType.max)
                        emit_tiny(t, s0, tmp)

            CHUNK = 512
            for t in range(1, T):
                step(t)
                if t % CHUNK == 0:
                    lo = (t - CHUNK) * NS
                    nc.sync.dma_start(
                        t1_out[:, lo:t * NS], t1hist[:, lo:t * NS])

            lo = (T // CHUNK * CHUNK - CHUNK) * NS
            nc.sync.dma_start(t1_out[:, lo:], t1hist[:, lo:])

    nc.finalize()
    return nc


def _get_nc():
    if "nc" not in _CACHE:
        _CACHE["nc"] = _build_forward()
    return _CACHE["nc"]


def kernel(transitions, emissions, lengths):
    from concourse.bass_utils import run_bass_kernel_spmd

    transitions = np.ascontiguousarray(transitions, dtype=np.float32)
    emissions = np.ascontiguousarray(emissions, dtype=np.float32)
    lengths = np.asarray(lengths, dtype=np.int32)
    assert transitions.shape == (B, S + 1, S)
    assert emissions.shape == (B, T, S)

    nc = _get_nc()
    eye = np.eye(S, dtype=np.float32)
    in_maps = [
        {
            "transitions": transitions[c * NS:(c + 1) * NS],
            "emissions": emissions[c * NS:(c + 1) * NS],
            "identity": eye,
        }
        for c in range(N_CORES)
    ]
    res = run_bass_kernel_spmd(
        nc, in_maps, core_ids=list(range(N_CORES)),
        trace=bool(os.environ.get("VIT_TRACE")),
    )
    if os.environ.get("VIT_TRACE"):
        _CACHE["last_exec_time_ns"] = res.exec_time_ns
        _CACHE["last_res"] = res

    t1 = np.empty((B, T, S), dtype=np.float32)
    for c in range(N_CORES):
        t1[c * NS:(c + 1) * NS] = (
            res.results[c]["t1hist"].reshape(S, T, NS).transpose(2, 1, 0)
        )

    return _backtrack(transitions, emissions, lengths, t1)


def _backtrack(transitions, emissions, lengths, t1):
    """Reference-exact backtrack from the t1 value history."""
    trans = transitions[:, :S, :]
    nb = np.arange(B)
    z = np.zeros((B, T), dtype=np.int32)
    last = lengths - 1
    z_last = np.argmax(t1[nb, last, :], axis=1).astype(np.int32)
    ptr = z_last.copy()
    for t in range(int(last.max()), 0, -1):
        at_last = (t == last)
        if at_last.any():
            ptr = np.where(at_last, z_last, ptr)
        z[:, t] = np.where(t <= last, ptr, 0)
        col = (t1[:, t - 1, :] + trans[nb, :, ptr]) + emissions[nb, t, ptr][:, None]
        ptr_new = np.argmax(col, axis=1).astype(np.int32)
        ptr = np.where(t <= last, ptr_new, ptr)
    z[:, 0] = ptr
    return z


# revision 22
# speedup vs baseline: 1.0475x; 1.0002x over previous
"""Batched Viterbi decode (CRF) on 8 Trainium2 NeuronCores.

Device computes the bit-exact fp32 t1 value history; host backtracks in
numpy (the em-after-max reorder is exact: max_j fl(fl(t1+A)+em) ==
fl(max_j fl(t1+A) + em) by monotonicity of rounding).

Per step, per quad-group of 4 seqs: bias-adds (A_s + t1col) on 2xACT +
1 fused GPSIMD pair-add, per-seq PE transpose into a PSUM quad, one DVE
max-reduce [128,512], one DVE em-add [128,4] into t1hist.  The two
groups' (reduce -> em-add) pairs share a single-buffered tmp tile: the
WAR dependency stops the Tile scheduler from batching both reduces
back-to-back on DVE (which would put group1's reduce on group0's
critical path) and keeps the groups anti-phased.  Transposes are queued
on the PE in add-engine FIFO-rank order so a late add never blocks an
earlier-ready transpose.  t1 history streams to DRAM in chunks.

Env knobs (defaults are the tuned config):
  V5_RED   reduce group size (4)
  V5_ADD   8-char engine string per seq: a=ACT, v=DVE tensor_scalar,
           g=GPSIMD ("aaggaagg"; adjacent g's in a group are fused)
  V5_EM    engine for the em-adds: v=DVE (default) or g=GPSIMD
  V5_F32R  2 = float32r-typed transpose path (default 0; the BIR
           verifier requires the whole producer chain typed fp32r)
"""

import os
from contextlib import ExitStack

import numpy as np

S = 128
T = 2048
NS = 8
N_CORES = 8
B = NS * N_CORES

RED = int(os.environ.get("V5_RED", "4"))
ADD_ENG = os.environ.get("V5_ADD", "aaggaagg")
EM_ENG = os.environ.get("V5_EM", "v")
# 0 = fp32 transposes; 2 = fp32r-typed transpose path (ident/sc/psum
# tensors declared float32r, engine reads/writes via fp32 bitcast)
F32R_MODE = int(os.environ.get("V5_F32R", "0"))
# fuse adjacent GPSIMD adds into one wide op (measured slower: the
# single op gates both transposes on its full completion)
GFUSE = os.environ.get("V5_GFUSE", "0") == "1"
# split each group's reduce: early 3-slice tensor_reduce + a fused
# tensor_tensor_reduce on the last-ready slice that adds em and writes
# the t1 column directly, unblocking the next step's adds earlier
TTR = os.environ.get("V5_TTR", "0") == "1"  # crashes NRT at runtime on this hw
SCBUFS = int(os.environ.get("V5_SCBUFS", "3"))

_CACHE = {}


def _build_forward():
    import concourse.bacc as bacc
    import concourse.mybir as mybir
    import concourse.tile as tile

    F32 = mybir.dt.float32
    F32R = mybir.dt.float32r
    nc = bacc.Bacc("TRN2", num_devices=N_CORES)
    trans_in = nc.dram_tensor("transitions", [NS, S + 1, S], F32, kind="ExternalInput")
    em_in = nc.dram_tensor("emissions", [NS, T, S], F32, kind="ExternalInput")
    ident_in = nc.dram_tensor("identity", [S, S], F32, kind="ExternalInput")
    t1_out = nc.dram_tensor("t1hist", [S, T * NS], F32, kind="ExternalOutput")

    NGRP = NS // RED  # reduce groups per step
    TPDT = F32R if F32R_MODE == 2 else F32

    with ExitStack() as ctx:
        trans_sb = ctx.enter_context(nc.sbuf_tensor([S, NS * S], F32))
        em_cols = ctx.enter_context(nc.sbuf_tensor([S, T * NS], F32))
        t1hist = ctx.enter_context(nc.sbuf_tensor([S, T * NS], F32))
        ident = ctx.enter_context(nc.sbuf_tensor([S, S], F32))
        start_sb = ctx.enter_context(nc.sbuf_tensor([S, NS], F32))
        em0_sb = ctx.enter_context(nc.sbuf_tensor([S, NS], F32))
        if F32R_MODE == 2:
            ident_r = ctx.enter_context(nc.sbuf_tensor([S, S], F32R))
        # 2 quad-sized PSUM tensors x 3-deep rotation = 6 banks; prologue
        # stage psum uses the remaining 2.
        psum_pp = [
            [
                ctx.enter_context(nc.psum_tensor(f"pspp{k}g{g}", [S, 4 * S], TPDT))
                for g in range(2)
            ]
            for k in range(3)
        ]

        with tile.TileContext(nc) as tc, ExitStack() as pctx:
            sc_pool = pctx.enter_context(tc.tile_pool(name="scores", bufs=SCBUFS))
            tmp_pool = pctx.enter_context(tc.tile_pool(name="tmp", bufs=1))
            stage_pool = pctx.enter_context(tc.tile_pool(name="stage", bufs=4))
            pst_pool = pctx.enter_context(tc.tile_pool(name="pst", bufs=2, space="PSUM"))

            # ---- prologue: transitions, identity, t1_0 ----
            for s in range(NS):
                nc.sync.dma_start(trans_sb[:, s * S:(s + 1) * S], trans_in[s, 0:S, :])
            nc.sync.dma_start(ident[:], ident_in[:])
            if F32R_MODE == 2:
                nc.sync.dma_start(ident_r[:], ident_in[:].bitcast(F32R))
            for s in range(NS):
                nc.sync.dma_start(
                    start_sb[:, s:s + 1], trans_in[s, S:S + 1, :].rearrange("o p -> p o")
                )
                nc.sync.dma_start(
                    em0_sb[:, s:s + 1], em_in[s, 0:1, :].rearrange("o p -> p o")
                )
            nc.vector.tensor_add(t1hist[:, 0:NS], start_sb[:], em0_sb[:])

            # ---- prologue: transpose emissions into em_cols[i, t*NS+s] ----
            for s in range(NS):
                for q in range(T // S // 4):
                    stage = stage_pool.tile([S, 4 * S], F32, tag="emstage")
                    pst = pst_pool.tile([S, 4 * S], F32, tag="empsum")
                    for k in range(4):
                        c = 4 * q + k
                        nc.sync.dma_start(
                            stage[:, k * S:(k + 1) * S], em_in[s, c * S:(c + 1) * S, :])
                        nc.tensor.transpose(
                            pst[:, k * S:(k + 1) * S], stage[:, k * S:(k + 1) * S],
                            ident[:])
                    dst = em_cols[:, 4 * q * S * NS + s: 4 * (q + 1) * S * NS: NS]
                    nc.scalar.copy(dst, pst[:])

            # ---- main DP loop ----
            def emit_add(s, t1col, dst):
                src = trans_sb[:, s * S:(s + 1) * S]
                e = ADD_ENG[s]
                if e == "a":
                    nc.scalar.activation(
                        dst, src, mybir.ActivationFunctionType.Identity,
                        bias=t1col, scale=1.0,
                    )
                elif e == "v":
                    nc.vector.tensor_scalar_add(dst, src, t1col)
                else:  # g
                    nc.gpsimd.tensor_add(
                        dst.rearrange("p (o i) -> p o i", i=S),
                        src.rearrange("p (o i) -> p o i", i=S),
                        t1col[:, :, None].to_broadcast([S, 1, S]))

            def emit_tiny(t, s0, tmp):
                # t1hist[:, t*NS+s0 : +RED] = u + em  (group granularity)
                lo = t * NS + s0
                args = (t1hist[:, lo:lo + RED], tmp[:],
                        em_cols[:, lo:lo + RED])
                if EM_ENG == "g":
                    nc.gpsimd.tensor_add(*args)
                else:
                    nc.vector.tensor_add(*args)

            def step(t):
                base = (t - 1) * NS
                for grp in range(NGRP):
                    s0 = grp * RED
                    q = (s0 // 4)
                    pst = psum_pp[t % 3][q]
                    half = (s0 % 4)
                    # emit ACT adds first: they serialize on the Scalar
                    # engine, so release them as early as possible
                    order = sorted(range(RED),
                                   key=lambda sl: ADD_ENG[s0 + sl] != "a")
                    # fuse two consecutive GPSIMD slots into one wide pair
                    # add: GPSIMD op cost is fixed-overhead dominated, so one
                    # [128,2,128] op beats two serialized [128,128] ops
                    gs = [sl for sl in order if ADD_ENG[s0 + sl] == "g"]
                    fused = ()
                    if (GFUSE and len(gs) == 2 and abs(gs[0] - gs[1]) == 1):
                        fused = (min(gs), max(gs))
                    sc_aps = {}
                    ranks = {}
                    nseen = {}
                    for sl in order:
                        s = s0 + sl
                        if fused and sl == fused[1]:
                            continue
                        if fused and sl == fused[0]:
                            scp = sc_pool.tile([S, 2 * S], TPDT, tag=f"scp{s}")
                            dstp = (scp[:].bitcast(F32)
                                    if F32R_MODE == 2 else scp[:])
                            nc.gpsimd.tensor_add(
                                dstp.rearrange("p (o i) -> p o i", i=S),
                                trans_sb[:, s * S:(s + 2) * S].rearrange(
                                    "p (o i) -> p o i", i=S),
                                t1hist[:, base + s:base + s + 2, None]
                                .to_broadcast([S, 2, S]))
                            sc_aps[fused[0]] = scp[:, 0:S]
                            sc_aps[fused[1]] = scp[:, S:2 * S]
                            ranks[fused[0]] = 0
                            ranks[fused[1]] = 0.5
                            continue
                        sc = sc_pool.tile([S, S], TPDT, tag=f"sc{s}")
                        dst = sc[:].bitcast(F32) if F32R_MODE == 2 else sc[:]
                        emit_add(s, t1hist[:, base + s:base + s + 1], dst)
                        sc_aps[sl] = sc[:]
                        e = ADD_ENG[s]
                        ranks[sl] = nseen.get(e, 0)
                        nseen[e] = ranks[sl] + 1
                    # transposes in expected ready-time order: a slot's add
                    # finishes after the other adds queued before it on the
                    # same engine, so interleave engines by FIFO rank to
                    # avoid head-of-line blocking in the PE queue
                    tp_order = sorted(order, key=lambda sl: ranks[sl])
                    for sl in tp_order:
                        nc.tensor.transpose(
                            pst[:, (half + sl) * S:(half + sl + 1) * S],
                            sc_aps[sl],
                            ident_r[:] if F32R_MODE == 2 else ident[:])
                    lo = t * NS + s0
                    if TTR and RED == 4 and tp_order[-1] == 3:
                        # early 3 slices: reduce + em-add as soon as the
                        # first three transposes land; the single-buffered
                        # tmp (WAR with the other group) keeps reduce->add
                        # adjacent on DVE and the groups anti-phased
                        pg3 = pst[:, half * S:(half + 3) * S]
                        if F32R_MODE == 2:
                            pg3 = pg3.bitcast(F32)
                        pg3 = pg3.rearrange("p (s i) -> p s i", i=S)
                        tmp = tmp_pool.tile([S, 3], F32, tag="u")
                        nc.vector.tensor_reduce(
                            tmp[:], pg3,
                            axis=mybir.AxisListType.X, op=mybir.AluOpType.max)
                        nc.vector.tensor_add(
                            t1hist[:, lo:lo + 3], tmp[:], em_cols[:, lo:lo + 3])
                        # last-ready slice: fused (psum + em) -> max writes
                        # the t1 column directly (reference rounding order)
                        p1 = pst[:, (half + 3) * S:(half + 4) * S]
                        if F32R_MODE == 2:
                            p1 = p1.bitcast(F32)
                        scr = sc_pool.tile([S, S], F32, tag=f"ttr{grp}")
                        nc.vector.tensor_tensor_reduce(
                            out=scr[:].rearrange("p (o i) -> p o i", i=S),
                            in0=p1.rearrange("p (o i) -> p o i", i=S),
                            in1=em_cols[:, lo + 3:lo + 4, None]
                            .to_broadcast([S, 1, S]),
                            scale=1.0, scalar=float(-3.4e38),
                            op0=mybir.AluOpType.add, op1=mybir.AluOpType.max,
                            accum_out=t1hist[:, lo + 3:lo + 4])
                    else:
                        pgsrc = pst[:, half * S:(half + RED) * S]
                        if F32R_MODE == 2:
                            pgsrc = pgsrc.bitcast(F32)
                        pg = pgsrc.rearrange("p (s i) -> p s i", i=S)
                        # single-buffered tmp shared by both groups: the WAR
                        # dependency forces the scheduler to keep each group's
                        # reduce->tiny adjacent on DVE instead of batching both
                        # reduces first (which puts group1's reduce on group0's
                        # critical path)
                        tmp = tmp_pool.tile([S, RED], F32, tag="u")
                        nc.vector.tensor_reduce(
                            tmp[:], pg,
                            axis=mybir.AxisListType.X, op=mybir.AluOpType.max)
                        emit_tiny(t, s0, tmp)

            CHUNK = 512
            for t in range(1, T):
                step(t)
                if t % CHUNK == 0:
                    lo = (t - CHUNK) * NS
                    nc.sync.dma_start(
                        t1_out[:, lo:t * NS], t1hist[:, lo:t * NS])

            lo = (T // CHUNK * CHUNK - CHUNK) * NS
            nc.sync.dma_start(t1_out[:, lo:], t1hist[:, lo:])

    nc.finalize()
    return nc


def _get_nc():
    if "nc" not in _CACHE:
        _CACHE["nc"] = _build_forward()
    return _CACHE["nc"]


def kernel(transitions, emissions, lengths):
    from concourse.bass_utils import run_bass_kernel_spmd

    transitions = np.ascontiguousarray(transitions, dtype=np.float32)
    emissions = np.ascontiguousarray(emissions, dtype=np.float32)
    lengths = np.asarray(lengths, dtype=np.int32)
    assert transitions.shape == (B, S + 1, S)
    assert emissions.shape == (B, T, S)

    nc = _get_nc()
    eye = np.eye(S, dtype=np.float32)
    in_maps = [
        {
            "transitions": transitions[c * NS:(c + 1) * NS],
            "emissions": emissions[c * NS:(c + 1) * NS],
            "identity": eye,
        }
        for c in range(N_CORES)
    ]
    res = run_bass_kernel_spmd(
        nc, in_maps, core_ids=list(range(N_CORES)),
        trace=bool(os.environ.get("VIT_TRACE")),
    )
    if os.environ.get("VIT_TRACE"):
        _CACHE["last_exec_time_ns"] = res.exec_time_ns
        _CACHE["last_res"] = res

    t1 = np.empty((B, T, S), dtype=np.float32)
    for c in range(N_CORES):
        t1[c * NS:(c + 1) * NS] = (
            res.results[c]["t1hist"].reshape(S, T, NS).transpose(2, 1, 0)
        )

    return _backtrack(transitions, emissions, lengths, t1)


def _backtrack(transitions, emissions, lengths, t1):
    """Reference-exact backtrack from the t1 value history."""
    trans = transitions[:, :S, :]
    nb = np.arange(B)
    z = np.zeros((B, T), dtype=np.int32)
    last = lengths - 1
    z_last = np.argmax(t1[nb, last, :], axis=1).astype(np.int32)
    ptr = z_last.copy()
    for t in range(int(last.max()), 0, -1):
        at_last = (t == last)
        if at_last.any():
            ptr = np.where(at_last, z_last, ptr)
        z[:, t] = np.where(t <= last, ptr, 0)
        col = (t1[:, t - 1, :] + trans[nb, :, ptr]) + emissions[nb, t, ptr][:, None]
        ptr_new = np.argmax(col, axis=1).astype(np.int32)
        ptr = np.where(t <= last, ptr_new, ptr)
    z[:, 0] = ptr
    return z
